# revision 1
# baseline (speedup 1.0000x reference)
"""InterfaceBoundaryLoss Trainium2 kernel.

Data-parallel over batch across 8 NeuronCores.  The [H,W] interface mask is
analyzed on the host and covered with a small set of rectangular "boxes";
the device only streams / computes the boxed regions (the mask is a thin
circle, so this is ~4% of the dense frame).  Per box, all 8 local batches
are fused into the free dimension of [rows, 8*w] tiles.

Math (per batch b, cell (i,j) with mask m=1):
  pot += (phi1-phi2)^2
  der += (EPS1*d1 - EPS2*d2)^2,  dk = nx*dpx_k + ny*dpy_k
Let psi = 0.025*phi2 - phi1 = -(80*phi1 - 2*phi2)/80.  Then
  EPS1*d1 - EPS2*d2 = -40000*(nx*Dx(psi) + ny*Dy(psi))
with Dx/Dy the raw central differences.  So with host fields
  A = 40000*m*nx, B = 40000*m*ny      (zero off-mask)
  der = sum((A*Dx(psi) + B*Dy(psi))^2)
Dy is computed on the TensorEngine via a banded +/-1 stationary matrix,
Dx on the VectorEngine via shifted views.  Square+sum reductions run on
the ScalarEngine (activation Square with accum_out); the pot path runs
on GpSimd.  Host sums per-partition partials in float64.

Mask cells on the frame border (edge-padding semantics) are computed
exactly on the host (none for the reference circle mask).
"""

import sys

for _p in ("/opt/trn_rl_repo",):
    if _p not in sys.path:
        sys.path.append(_p)

import numpy as np
import ml_dtypes

B, H, W = 64, 1024, 1024
EPS1, EPS2 = 80.0, 2.0
DX, DY = 0.001, 0.001
CX, CY = 512.0, 512.0
WEIGHT = 1.0
N_CORES = 8
BPC = B // N_CORES

# "bf16" or "f32" device compute dtype.
DEVICE_DTYPE = "f32"

# set TRACE=True (e.g. from a test harness) to profile the device run;
# LAST_EXEC_NS then holds the measured NEFF execution time.
TRACE = False
LAST_EXEC_NS = None

_FD_CAP = {"f32": 2048, "bf16": 4096}


def _normals(h, w):
    ii = np.arange(h, dtype=np.float64)[:, None]
    jj = np.arange(w, dtype=np.float64)[None, :]
    nx = jj - CX
    ny = ii - CY
    norm = np.sqrt(nx * nx + ny * ny)
    safe = np.where(norm > 0, norm, 1.0)
    return nx / safe, ny / safe


def _cluster(cols, gap):
    """Group sorted col indices into (start, end) inclusive intervals."""
    out = []
    s = p = cols[0]
    for c in cols[1:]:
        if c - p > gap:
            out.append((s, p))
            s = c
        p = c
    out.append((s, p))
    return out


class _Box:
    __slots__ = ("r0", "nrows", "c0", "w", "nb", "ngroups")

    def __init__(self, r0, nrows, c0, w):
        self.r0, self.nrows, self.c0, self.w = int(r0), int(nrows), int(c0), int(w)


def _plan(mask):
    """Cover interior mask cells with boxes.

    Each box loads rows [r0, r0+nrows) x cols [c0, c0+w); cells assigned to
    it are in relative rows [1, nrows-2] and relative cols [1, w-2].
    Returns (boxes, assigned_masks, host_cells) where assigned_masks is the
    per-box bool array [nrows, w] of cells this box owns.
    """
    h, w_ = mask.shape
    border = np.zeros_like(mask)
    border[0, :] = border[-1, :] = True
    border[:, 0] = border[:, -1] = True
    host_cells = mask & border
    core = mask & ~border

    # Recursive cost-driven segmentation: a segment of rows is covered by
    # one box per column-cluster; split the segment in half whenever the
    # two halves' covers are cheaper (box fixed cost ~3000 cyc, ~22 cyc/col).
    def seg_specs(rs, h):
        cols = np.flatnonzero(core[rs : rs + h].any(axis=0))
        if len(cols) == 0:
            return 0.0, []
        clusters = _cluster(cols, gap=17)
        if len(clusters) > 2:
            clusters = [(cols[0], cols[-1])]
        cost = sum(3000.0 + 22.0 * (cb - ca + 10) for ca, cb in clusters)
        return cost, [(rs, h, clusters)]

    def dp(rs, h):
        c0, s0 = seg_specs(rs, h)
        if h <= 2 or not s0:
            return c0, s0
        h1 = h // 2
        ca_, sa = dp(rs, h1)
        cb_, sb = dp(rs + h1, h - h1)
        if ca_ + cb_ < c0:
            return ca_ + cb_, sa + sb
        return c0, s0

    boxes = []
    owned = []
    assigned = np.zeros_like(mask)
    rows_any = np.flatnonzero(core.any(axis=1))
    if len(rows_any):
        r = rows_any[0]
        rmax = rows_any[-1]
        specs = []
        while r <= rmax:
            if not core[r].any():
                r += 1
                continue
            h0 = min(126, rmax + 1 - r)
            _, s = dp(r, h0)
            specs.extend(s)
            r += h0
        for rs, hseg, clusters in specs:
            re_ = rs + hseg
            r = rs
            for ca0, cb0 in clusters:
                # split clusters wider than 498 so box width stays <= 512
                for ca in range(ca0, cb0 + 1, 498):
                    cb = min(ca + 497, cb0)
                    c0 = ca - 2
                    bw = cb + 3 - c0
                    if c0 % 2:
                        c0 -= 1
                        bw += 1
                    bw = -(-bw // 8) * 8
                    if c0 < 0:
                        c0 = 0
                    if c0 + bw > w_:
                        c0 = w_ - bw
                    bx = _Box(r - 1, hseg + 2, c0, bw)
                    sel = np.zeros((bx.nrows, bw), dtype=bool)
                    sub = core[r:re_, ca : cb + 1] & ~assigned[r:re_, ca : cb + 1]
                    sel[1 : 1 + hseg, ca - c0 : cb + 1 - c0] = sub
                    assigned[r:re_, ca : cb + 1] |= sub
                    boxes.append(bx)
                    owned.append(sel)
            r = re_

    leftover = core & ~assigned
    if leftover.any():
        host_cells = host_cells | leftover
        for sel, bx in zip(owned, boxes):
            lv = leftover[bx.r0 : bx.r0 + bx.nrows, bx.c0 : bx.c0 + bx.w]
            sel &= ~lv
    return boxes, owned, host_cells


def _host_contrib(cells_ij, phi1, phi2, nx, ny):
    """Exact (edge-padded) pot/der sums for the given cells, all batches."""
    if len(cells_ij[0]) == 0:
        return 0.0, 0.0
    ii, jj = cells_ij
    p1 = phi1.astype(np.float64)
    p2 = phi2.astype(np.float64)
    d = p1[:, ii, jj] - p2[:, ii, jj]
    pot = float(np.sum(d * d))

    # edge-padded central differences: clamp the *derivative* index
    jc = np.clip(jj, 1, W - 2)
    ic = np.clip(ii, 1, H - 2)

    def dn(p):
        dpx = (p[:, ii, jc + 1] - p[:, ii, jc - 1]) / (2.0 * DX)
        dpy = (p[:, ic + 1, jj] - p[:, ic - 1, jj]) / (2.0 * DY)
        return nx[ii, jj] * dpx + ny[ii, jj] * dpy

    mm = EPS1 * dn(p1) - EPS2 * dn(p2)
    der = float(np.sum(mm * mm))
    return pot, der


def _build_nc(boxes, dt_str, fd_cap):
    from contextlib import ExitStack
    from concourse import bass, bacc, tile, mybir

    mdt = mybir.dt.bfloat16 if dt_str == "bf16" else mybir.dt.float32
    f32 = mybir.dt.float32
    mult = mybir.AluOpType.mult
    sub = mybir.AluOpType.subtract
    SQ = mybir.ActivationFunctionType.Square

    njobs = sum(bx.ngroups for bx in boxes)
    nc = bacc.Bacc(
        "TRN2", target_bir_lowering=False, debug=False, num_devices=N_CORES
    )

    phi1_d = nc.dram_tensor("phi1", [BPC * H, W], mdt, kind="ExternalInput")
    phi2_d = nc.dram_tensor("phi2", [BPC * H, W], mdt, kind="ExternalInput")
    dmat_d = nc.dram_tensor("dmat", [128, 128], mdt, kind="ExternalInput")
    a_ds, b_ds, m_ds = [], [], []
    for k, bx in enumerate(boxes):
        a_ds.append(nc.dram_tensor(f"a{k}", [bx.nrows, bx.w], mdt, kind="ExternalInput"))
        b_ds.append(nc.dram_tensor(f"b{k}", [bx.nrows, bx.w], mdt, kind="ExternalInput"))
        m_ds.append(nc.dram_tensor(f"m{k}", [bx.nrows, bx.w], mdt, kind="ExternalInput"))
    acc_d = nc.dram_tensor("acc", [128, 2 * njobs], f32, kind="ExternalOutput")

    with tile.TileContext(nc) as tc, ExitStack() as ctx:
        io = ctx.enter_context(tc.tile_pool(name="io", bufs=3))
        cst = ctx.enter_context(tc.tile_pool(name="cst", bufs=2))
        mid = ctx.enter_context(tc.tile_pool(name="mid", bufs=2))
        pot_p = ctx.enter_context(tc.tile_pool(name="potp", bufs=2))
        psum = ctx.enter_context(tc.tile_pool(name="psum", bufs=4, space="PSUM"))
        onep = ctx.enter_context(tc.tile_pool(name="onep", bufs=1))

        dm = onep.tile([128, 128], mdt)
        nc.sync.dma_start(dm[:], dmat_d.ap())
        acc = onep.tile([128, 2 * njobs], f32)
        nc.vector.memset(acc[:], 0.0)

        job = 0
        for k, bx in enumerate(boxes):
            nr, w, nb = bx.nrows, bx.w, bx.nb
            fd = nb * w
            at = cst.tile([nr, w], mdt, tag="at")
            nc.sync.dma_start(at[:], a_ds[k].ap())
            bt = cst.tile([nr, w], mdt, tag="bt")
            nc.sync.dma_start(bt[:], b_ds[k].ap())
            mt = cst.tile([nr, w], mdt, tag="mt")
            nc.sync.dma_start(mt[:], m_ds[k].ap())
            a3 = at[:].unsqueeze(1).broadcast_to([nr, nb, w])
            b3 = bt[:].unsqueeze(1).broadcast_to([nr, nb, w])
            m3 = mt[:].unsqueeze(1).broadcast_to([nr, nb, w])

            for g in range(bx.ngroups):
                b0 = g * nb
                f1 = io.tile([nr, fd], mdt, tag="f1")
                f2 = io.tile([nr, fd], mdt, tag="f2")
                for ft, src_d in ((f1, phi1_d), (f2, phi2_d)):
                    src = bass.AP(
                        src_d,
                        (b0 * H + bx.r0) * W + bx.c0,
                        [[W, nr], [H * W, nb], [1, w]],
                    )
                    dst = ft[:].rearrange("p (b w) -> p b w", b=nb)
                    nc.sync.dma_start(dst, src)

                # psi = 0.025*phi2 - phi1
                psi = mid.tile([nr, fd], mdt, tag="psi")
                nc.vector.scalar_tensor_tensor(
                    psi[:], f2[:], 0.025, f1[:], op0=mult, op1=sub
                )

                # dxs[f] = psi[f+2] - psi[f]  (cell at f+1)
                dxs = mid.tile([nr, fd], mdt, tag="dxs")
                nc.vector.tensor_sub(
                    dxs[:, 0 : fd - 2], psi[:, 2:fd], psi[:, 0 : fd - 2]
                )
                nc.vector.memset(dxs[:, fd - 2 : fd], 0.0)
                u = mid.tile([nr, fd], mdt, tag="u")
                nc.vector.tensor_mul(
                    u[:].rearrange("p (b w) -> p b w", b=nb),
                    dxs[:].rearrange("p (b w) -> p b w", b=nb),
                    a3,
                )

                # dy via PE: dy[mi, f] = psi[mi+1, f] - psi[mi-1, f];
                # batch-aligned chunks of gchunk blocks (gchunk*w <= 512)
                v = mid.tile([nr, fd], mdt, tag="v")
                v3 = v[:].rearrange("p (b w) -> p b w", b=nb)
                nc.vector.memset(v3[:, :, w - 1 : w], 0.0)
                gchunk = max(1, 512 // w)
                for j0 in range(0, nb, gchunk):
                    gg = min(gchunk, nb - j0)
                    dy = psum.tile([128, 512], f32, tag="dy")
                    nc.tensor.matmul(
                        dy[:, 0 : gg * w],
                        dm[0:nr, :],
                        psi[:, j0 * w : (j0 + gg) * w],
                        start=True,
                        stop=True,
                    )
                    dy3 = dy[0:nr, 0 : gg * w].rearrange("p (b w) -> p b w", b=gg)
                    nc.vector.tensor_mul(
                        v3[:, j0 : j0 + gg, 0 : w - 1],
                        bt[:].unsqueeze(1).broadcast_to([nr, gg, w])[:, :, 0 : w - 1],
                        dy3[:, :, 1:w],
                    )

                wt = mid.tile([nr, fd], mdt, tag="wt")
                nc.vector.tensor_add(wt[:], u[:], v[:])
                nc.scalar.activation(
                    dxs[:],
                    wt[:],
                    SQ,
                    accum_out=acc[0:nr, njobs + job : njobs + job + 1],
                )

                # pot path on GpSimd
                df = pot_p.tile([nr, fd], mdt, tag="df")
                nc.gpsimd.tensor_sub(df[:], f2[:], f1[:])
                w1 = pot_p.tile([nr, fd], mdt, tag="w1")
                nc.gpsimd.tensor_mul(
                    w1[:].rearrange("p (b w) -> p b w", b=nb),
                    df[:].rearrange("p (b w) -> p b w", b=nb),
                    m3,
                )
                nc.scalar.activation(
                    df[:],
                    w1[:],
                    SQ,
                    accum_out=acc[0:nr, job : job + 1],
                )
                job += 1

        nc.sync.dma_start(acc_d.ap(), acc[:])

    nc.compile()
    return nc


def _prepare(mask):
    """Plan boxes and build all mask-derived constant arrays."""
    nx, ny = _normals(H, W)
    boxes, owned, host_cells = _plan(mask)

    fd_cap = _FD_CAP[DEVICE_DTYPE]
    np_dt = ml_dtypes.bfloat16 if DEVICE_DTYPE == "bf16" else np.float32

    for bx in boxes:
        nb = max(1, min(BPC, fd_cap // bx.w))
        while BPC % nb:
            nb -= 1
        bx.nb = nb
        bx.ngroups = BPC // nb

    consts = {}
    af = 40000.0 * nx
    bf = 40000.0 * ny
    for k, (bx, sel) in enumerate(zip(boxes, owned)):
        rs, cs = slice(bx.r0, bx.r0 + bx.nrows), slice(bx.c0, bx.c0 + bx.w)
        a_box = np.where(sel, af[rs, cs], 0.0)
        b_box = np.where(sel, bf[rs, cs], 0.0)
        # shift left by one col: field[k] = value at col k+1
        a_sh = np.zeros_like(a_box)
        a_sh[:, :-1] = a_box[:, 1:]
        b_sh = np.zeros_like(b_box)
        b_sh[:, :-1] = b_box[:, 1:]
        consts[f"a{k}"] = a_sh.astype(np_dt)
        consts[f"b{k}"] = b_sh.astype(np_dt)
        consts[f"m{k}"] = sel.astype(np_dt)

    dmat = np.zeros((128, 128), dtype=np.float64)
    for mi in range(1, 127):
        dmat[mi + 1, mi] = 1.0
        dmat[mi - 1, mi] = -1.0
    consts["dmat"] = dmat.astype(np_dt)
    return boxes, consts, host_cells, np_dt


_CACHE = {}


def kernel(output_in, output_out, interface_mask):
    from concourse.bass_utils import run_bass_kernel_spmd

    phi1 = np.asarray(output_in).reshape(B, H, W)
    phi2 = np.asarray(output_out).reshape(B, H, W)
    mask = np.asarray(interface_mask).astype(bool)

    n_mask = float(mask.sum())
    if n_mask == 0.0:
        return np.float32(np.nan)

    key = (mask.tobytes(), DEVICE_DTYPE)
    if key not in _CACHE:
        boxes, consts, host_cells, np_dt = _prepare(mask)
        nc = _build_nc(boxes, DEVICE_DTYPE, _FD_CAP[DEVICE_DTYPE]) if boxes else None
        _CACHE[key] = (boxes, consts, host_cells, np_dt, nc)
    boxes, consts, host_cells, np_dt, nc = _CACHE[key]

    pot = der = 0.0
    if nc is not None:
        in_maps = []
        for c in range(N_CORES):
            sl = slice(c * BPC, (c + 1) * BPC)
            m = dict(consts)
            m["phi1"] = np.ascontiguousarray(phi1[sl]).reshape(BPC * H, W).astype(np_dt)
            m["phi2"] = np.ascontiguousarray(phi2[sl]).reshape(BPC * H, W).astype(np_dt)
            in_maps.append(m)
        res = run_bass_kernel_spmd(
            nc, in_maps, core_ids=list(range(N_CORES)), trace=TRACE
        )
        global LAST_EXEC_NS
        LAST_EXEC_NS = res.exec_time_ns
        njobs = sum(bx.ngroups for bx in boxes)
        for r in res.results:
            a = r["acc"].astype(np.float64)
            pot += float(a[:, :njobs].sum())
            der += float(a[:, njobs:].sum())

    if host_cells.any():
        nx, ny = _normals(H, W)
        hp, hd = _host_contrib(np.nonzero(host_cells), phi1, phi2, nx, ny)
        pot += hp
        der += hd

    denom = B * n_mask
    return np.float32(WEIGHT * (pot + der) / denom)



# revision 13
# speedup vs baseline: 3.2105x; 3.2105x over previous
"""InterfaceBoundaryLoss Trainium2 kernel.

Data-parallel over batch across 8 NeuronCores; each core owns 8 batches.

The [H,W] interface mask (a thin circle) is analyzed on the host and covered
with "segments": sheared bands of K*8 consecutive image rows whose column
window slides by an integer shear per 8-row group (vertical segments), or
K column-chunks of one 8-row band (horizontal segments, for flat/wide
stretches).  Either way one segment is ONE strided DMA whose destination is
a plain [K*8 partitions, 8w cols] SBUF slice.  Segments are stacked into the
128 partitions; all segments together form a single packed [128, E] layout
(E = packed_cols * 8 batches, batch innermost).  Host uploads phi in
batch-inner [H, W, 8] layout so each DMA line covers all 8 batches.

All compute runs as a handful of full-width instructions:
  psi  = 0.025*phi2 - phi1                       (Vector, one stt)
  dxs  = psi[x+1] - psi[x-1]   (shifted sub)     (Vector, one sub)
  dy   = banded 128x128 matmul over partitions   (PE, E/512 matmuls)
  u    = A_sh * dxs ; v = B * dy ; wt = u + v>>1 (Vector)
  df   = phi1 - phi2 ; w1 = M * df               (GpSimd)
  der  = sum(wt^2), pot = sum(w1^2)              (Scalar activation accum)
Segment margins carry A=B=M=0 so stencil reads crossing segment/slot
boundaries are multiplied by zero; every element of the packed region is
either real (finite) DRAM data or memset, so no NaNs reach the matmul.
Work is split into two halves so compute overlaps the tail DMAs.

Mask cells on the frame border (edge-padding semantics) and any cells the
planner cannot cover are computed exactly on the host (none for the circle).
"""

import sys

for _p in ("/opt/trn_rl_repo",):
    if _p not in sys.path:
        sys.path.append(_p)

import numpy as np
import ml_dtypes

B, H, W = 64, 1024, 1024
EPS1, EPS2 = 80.0, 2.0
DX, DY = 0.001, 0.001
CX, CY = 512.0, 512.0
WEIGHT = 1.0
N_CORES = 8
BPC = B // N_CORES  # 8 batches per core, innermost in device layout

# "bf16" or "f32" device compute dtype.
DEVICE_DTYPE = "bf16"

# set TRACE=True (e.g. from a test harness) to profile the device run;
# LAST_EXEC_NS then holds the measured NEFF execution time.
TRACE = False
LAST_EXEC_NS = None

GR = 8            # rows loaded per box (partition granularity)
OSTEP = 6         # owned rows per vertical box (boxes overlap by 2)
ROW_GAP = 8       # col gap that splits a row into two intervals
CHAIN_SLACK = 24  # max col distance to keep chaining rows
CHOPW = 28        # box window wider than this -> horizontal chop
WCHUNK = 16       # owned cols per horizontal chunk


def _normals(h, w):
    ii = np.arange(h, dtype=np.float64)[:, None]
    jj = np.arange(w, dtype=np.float64)[None, :]
    nx = jj - CX
    ny = ii - CY
    norm = np.sqrt(nx * nx + ny * ny)
    safe = np.where(norm > 0, norm, 1.0)
    return nx / safe, ny / safe


def _intervals(cols, gap):
    out = []
    s = p = cols[0]
    for c in cols[1:]:
        if c - p > gap:
            out.append((s, p))
            s = c
        p = c
    out.append((s, p))
    return out


class _Seg:
    """K stacked 8-row windows (boxes), one DMA.

    vertical (rstep=OSTEP): box g loads rows [r0+6g, r0+6g+8),
        cols [c0+g*dlt, +w).  Box g owns rows [r0+6g+1, r0+6g+7).
    horizontal (rstep=0): all boxes load rows [r0, r0+8),
        box g cols [c0+g*dlt, +w).  Owned rows [r0+1, r0+7).
    Every box is vertically self-contained (Dy stencil stays inside its
    8 loaded rows), so the per-box column window may shear freely.
    """
    __slots__ = ("r0", "K", "c0", "dlt", "w", "rstep", "rows", "slot",
                 "s0", "off", "w_load", "half")

    def __init__(self, r0, K, c0, dlt, w, rstep, rows):
        self.r0, self.K, self.c0 = int(r0), int(K), int(c0)
        self.dlt, self.w, self.rstep = int(dlt), int(w), int(rstep)
        self.rows = rows  # dict row -> (lo, hi) owned cells of this seg


def _plan(mask):
    """Cover interior mask cells with segments."""
    h, w_ = mask.shape
    border = np.zeros_like(mask)
    border[0, :] = border[-1, :] = True
    border[:, 0] = border[:, -1] = True
    host_cells = mask & border
    core = mask & ~border

    rows_any = np.flatnonzero(core.any(axis=1))
    if rows_any.size == 0:
        return [], host_cells

    # per-row intervals
    row_iv = {}
    for r in rows_any:
        cols = np.flatnonzero(core[r])
        for iv in _intervals(cols, ROW_GAP):
            row_iv.setdefault(r, []).append(iv)

    # chains: link intervals across consecutive rows
    chains = []      # each: list of (r, lo, hi)
    active = {}      # chain idx -> (r, lo, hi)
    for r in rows_any:
        newactive = {}
        used = set()
        for (lo, hi) in row_iv[r]:
            best, bestov = None, None
            for ci, (pr, plo, phi) in active.items():
                if ci in used or pr != r - 1:
                    continue
                ov = min(hi, phi) - max(lo, plo)
                if ov >= -CHAIN_SLACK and (bestov is None or ov > bestov):
                    best, bestov = ci, ov
            if best is None:
                chains.append([])
                best = len(chains) - 1
            chains[best].append((int(r), int(lo), int(hi)))
            used.add(best)
            newactive[best] = (int(r), int(lo), int(hi))
        active = newactive

    segs = []

    def chop(r0, rows):
        """Horizontal chop of rows [r0+1, r0+7) into WCHUNK-col chunks."""
        allc = sorted(set(c for (lo, hi) in rows.values() for c in (lo, hi)))
        lo = min(lo for (lo, hi) in rows.values())
        hi = max(hi for (lo, hi) in rows.values())
        cw = hi - lo + 1
        K = -(-cw // WCHUNK)
        out = []
        for k0 in range(0, K, 16):
            kk = min(16, K - k0)
            rws = {}
            for r, (l, hh) in rows.items():
                a = max(l, lo + k0 * WCHUNK)
                b = min(hh, lo + (k0 + kk) * WCHUNK - 1)
                if a <= b:
                    rws[r] = (a, b)
            out.append(_Seg(r0, kk, lo + k0 * WCHUNK - 1, WCHUNK,
                            WCHUNK + 2, 0, rws))
        return out

    def win_fit(gwins):
        K = len(gwins)
        good = [i for i, wn in enumerate(gwins) if wn is not None]
        los = np.array([gwins[i][0] for i in good], dtype=np.int64)
        his = np.array([gwins[i][1] for i in good], dtype=np.int64)
        if len(good) > 1:
            d = int(round((los[-1] - los[0]) / (good[-1] - good[0])))
        else:
            d = 0
        gi = np.array(good)
        c0 = int((los - 1 - gi * d).min())
        w = int((his + 2 - (c0 + gi * d)).max() + 1)
        return c0, d, w

    def seg_windows(ch_rows, lo_r, hi_r):
        """Box windows for owned rows [lo_r, hi_r); box g owns 6 rows
        [lo_r + 6g, +6) and loads them plus one margin row above/below."""
        K = -(-(hi_r - lo_r) // OSTEP)
        wins = []
        for g in range(K):
            lo = hi = None
            ra = lo_r + OSTEP * g
            for r in range(ra, min(ra + OSTEP, hi_r)):
                iv = ch_rows.get(r)
                if iv is None:
                    continue
                lo = iv[0] if lo is None else min(lo, iv[0])
                hi = iv[1] if hi is None else max(hi, iv[1])
            wins.append(None if lo is None else (lo, hi))
        return K, wins

    def segcost(w, K):
        return 600.0 + 40.0 * w * max(K, 4) / 16.0

    def emit(ch_rows, lo_r, hi_r):
        K, wins = seg_windows(ch_rows, lo_r, hi_r)
        c0, d, w = win_fit(wins)
        nown = hi_r - lo_r
        if K == 1:
            if w > CHOPW + 4:
                rws = {r: ch_rows[r] for r in range(lo_r, hi_r)
                       if r in ch_rows}
                return chop(lo_r - 1, rws)
            return [_Seg(lo_r - 1, K, c0, d, w, OSTEP,
                         {r: ch_rows[r] for r in range(lo_r, hi_r)
                          if r in ch_rows})]
        # try split at the middle box boundary
        mid = lo_r + OSTEP * (K // 2)
        Kl, wl_ = seg_windows(ch_rows, lo_r, mid)
        Kr, wr_ = seg_windows(ch_rows, mid, hi_r)
        _, _, wl = win_fit(wl_)
        _, _, wr = win_fit(wr_)
        if (w > CHOPW + 4) or (segcost(wl, Kl) + segcost(wr, Kr)
                               < segcost(w, K)):
            return emit(ch_rows, lo_r, mid) + emit(ch_rows, mid, hi_r)
        return [_Seg(lo_r - 1, K, c0, d, w, OSTEP,
                     {r: ch_rows[r] for r in range(lo_r, hi_r)
                      if r in ch_rows})]

    for ch in chains:
        ch_rows = {r: (lo, hi) for (r, lo, hi) in ch}
        rs = sorted(ch_rows)
        start = 0
        for i in range(1, len(rs) + 1):
            if i == len(rs) or rs[i] != rs[i - 1] + 1:
                lo_r, hi_r = rs[start], rs[i - 1] + 1
                while lo_r < hi_r:
                    top = min(lo_r + 16 * OSTEP, hi_r)
                    segs.extend(emit(ch_rows, lo_r, top))
                    lo_r = top
                start = i
    return segs, host_cells


def _host_contrib(cells_ij, phi1, phi2, nx, ny):
    if len(cells_ij[0]) == 0:
        return 0.0, 0.0
    ii, jj = cells_ij
    p1 = phi1.astype(np.float64)
    p2 = phi2.astype(np.float64)
    d = p1[:, ii, jj] - p2[:, ii, jj]
    pot = float(np.sum(d * d))
    jc = np.clip(jj, 1, W - 2)
    ic = np.clip(ii, 1, H - 2)

    def dn(p):
        dpx = (p[:, ii, jc + 1] - p[:, ii, jc - 1]) / (2.0 * DX)
        dpy = (p[:, ic + 1, jj] - p[:, ic - 1, jj]) / (2.0 * DY)
        return nx[ii, jj] * dpx + ny[ii, jj] * dpy

    mm = EPS1 * dn(p1) - EPS2 * dn(p2)
    der = float(np.sum(mm * mm))
    return pot, der


def _prepare(mask):
    """Plan the cover and build packed const arrays + DMA specs."""
    segs, host_cells = _plan(mask)
    np_dt = ml_dtypes.bfloat16 if DEVICE_DTYPE == "bf16" else np.float32
    if not segs:
        return None, host_cells, np_dt

    nx, ny = _normals(H, W)
    af = 40000.0 * nx
    bf = 40000.0 * ny

    # shelf-pack into slots (16 stacks of GR partitions)
    slots = []  # [span, stacks_used, [segs]]
    drop = []
    for s in sorted(segs, key=lambda s: -s.w):
        placed = False
        for si, sl in enumerate(slots):
            if sl[1] + s.K <= 16 and s.w <= sl[0]:
                s.slot = si
                s.s0 = sl[1]
                sl[1] += s.K
                sl[2].append(s)
                placed = True
                break
        if not placed:
            s.slot = len(slots)
            s.s0 = 0
            slots.append([s.w, s.K, [s]])

    Wp = sum(sl[0] for sl in slots)
    off = 0
    for sl in slots:
        for s in sl[2]:
            s.off = off
            s.w_load = sl[0]
        off += sl[0]

    # halves split at a slot boundary near Wp/2
    acc = 0
    half_split = len(slots)
    for si, sl in enumerate(slots):
        if acc >= Wp / 2:
            half_split = si
            break
        acc += sl[0]
    wp1 = sum(sl[0] for sl in slots[:half_split])
    if wp1 in (0, Wp):
        wp1 = Wp
    for si, sl in enumerate(slots):
        for s in sl[2]:
            s.half = 0 if si < half_split else 1

    A = np.zeros((128, Wp), dtype=np.float64)
    Bc = np.zeros((128, Wp), dtype=np.float64)
    M = np.zeros((128, Wp), dtype=np.float64)
    filled = np.zeros((128, Wp), dtype=bool)

    borderm = np.zeros_like(mask)
    borderm[0, :] = borderm[-1, :] = True
    borderm[:, 0] = borderm[:, -1] = True
    core = mask & ~borderm

    extra_host = np.zeros_like(mask)
    dma_specs = []  # (half, src_off, src_dims, p0, P, col0_e, ncol_e)
    for s in segs:
        # bounds clamp for extended load width
        cmax = max(s.c0 + g * s.dlt for g in range(s.K))
        cmin = min(s.c0 + g * s.dlt for g in range(s.K))
        shift = 0
        if cmax + s.w_load > W:
            shift = cmax + s.w_load - W
        if cmin - shift < 0:
            # cannot satisfy bounds: push cells to host
            for r, (lo, hi) in s.rows.items():
                extra_host[r, lo:hi + 1] = True
            continue
        s.c0 -= shift

        rmax_load = s.r0 + (s.K - 1) * s.rstep + GR
        assert s.r0 >= 0 and rmax_load <= H

        ok = True
        if s.rstep != 0:
            for r, (lo, hi) in s.rows.items():
                g = (r - s.r0 - 1) // OSTEP
                cg = s.c0 + g * s.dlt
                if not (0 <= g < s.K and cg + 1 <= lo
                        and hi <= cg + s.w - 2):
                    ok = False
                    break
        if not ok:
            for r, (lo, hi) in s.rows.items():
                extra_host[r, lo:hi + 1] = True
            continue

        for r, (lo, hi) in s.rows.items():
            if s.rstep == 0:
                gs = range(s.K)
                rel = r - s.r0
                assert 1 <= rel <= GR - 2
            else:
                gs = [(r - s.r0 - 1) // OSTEP]
            for g in gs:
                cg = s.c0 + g * s.dlt
                if s.rstep != 0:
                    rel = r - (s.r0 + g * OSTEP)
                    assert 1 <= rel <= GR - 2
                llo, lhi = max(lo, cg + 1), min(hi, cg + s.w - 2)
                for c in range(llo, lhi + 1):
                    if not core[r, c]:
                        continue
                    p = (s.s0 + g) * GR + rel
                    x = s.off + (c - cg)
                    assert not filled[p, x]
                    filled[p, x] = True
                    M[p, x] = 1.0
                    Bc[p, x] = bf[r, c]
                    A[p, x - 1] = af[r, c]

        sstep = (s.rstep * W + s.dlt) * BPC
        src_off = (s.r0 * W + s.c0) * BPC
        if s.K > 1:
            src_dims = [[sstep, s.K], [BPC * W, GR], [1, BPC * s.w_load]]
        else:
            src_dims = [[BPC * W, GR], [1, BPC * s.w_load]]
        dma_specs.append((s.half, src_off, src_dims, s.s0 * GR, s.K * GR,
                          s.off * BPC, s.w_load * BPC))

    if extra_host.any():
        host_cells = host_cells | extra_host

    # memset unfilled stacks of each slot
    memset_specs = []
    off = 0
    for si, sl in enumerate(slots):
        span, used, _ = sl
        if used < 16:
            memset_specs.append((0 if si < half_split else 1,
                                 used * GR, 128, off * BPC, span * BPC))
        off += span

    consts = np.concatenate([A, Bc, M], axis=1).astype(np_dt)
    dmat = np.zeros((128, 128), dtype=np.float64)
    for mi in range(1, 127):
        dmat[mi + 1, mi] = 1.0
        dmat[mi - 1, mi] = -1.0

    plan = {
        "Wp": Wp, "wp1": wp1, "E": Wp * BPC, "E1": wp1 * BPC,
        "dma_specs": dma_specs, "memset_specs": memset_specs,
        "consts": consts, "dmat": dmat.astype(np_dt), "segs": segs,
    }
    return plan, host_cells, np_dt


def _build_nc(plan, dt_str):
    from contextlib import ExitStack
    from concourse import bass, bacc, tile, mybir

    mdt = mybir.dt.bfloat16 if dt_str == "bf16" else mybir.dt.float32
    f32 = mybir.dt.float32
    mult = mybir.AluOpType.mult
    subt = mybir.AluOpType.subtract
    SQ = mybir.ActivationFunctionType.Square

    Wp, E, E1 = plan["Wp"], plan["E"], plan["E1"]
    halves = [(0, E1), (E1, E)] if E1 not in (0, E) else [(0, E)]

    nc = bacc.Bacc(
        "TRN2", target_bir_lowering=False, debug=False, num_devices=N_CORES
    )
    p1d = nc.dram_tensor("p1", [H * W * BPC], mdt, kind="ExternalInput")
    p2d = nc.dram_tensor("p2", [H * W * BPC], mdt, kind="ExternalInput")
    cst_d = nc.dram_tensor("cst", [128, 3 * Wp], mdt, kind="ExternalInput")
    dmat_d = nc.dram_tensor("dmat", [128, 128], mdt, kind="ExternalInput")
    acc_d = nc.dram_tensor("acc", [128, 8], f32, kind="ExternalOutput")

    with tile.TileContext(nc) as tc, ExitStack() as ctx:
        io = ctx.enter_context(tc.tile_pool(name="io", bufs=1))
        psum = ctx.enter_context(tc.tile_pool(name="ps", bufs=4, space="PSUM"))

        f1 = io.tile([128, E], mdt, tag="f1")
        f2 = io.tile([128, E], mdt, tag="f2")
        psi = io.tile([128, E], mdt, tag="psi")
        dxs = io.tile([128, E], mdt, tag="dxs")
        u = io.tile([128, E], mdt, tag="u")
        v = io.tile([128, E], mdt, tag="v")
        wt = io.tile([128, E], mdt, tag="wt")
        df = io.tile([128, E], mdt, tag="df")
        w1 = io.tile([128, E], mdt, tag="w1")
        scr = io.tile([128, E], mdt, tag="scr")
        cst = io.tile([128, 3 * Wp], mdt, tag="cst")
        dm = io.tile([128, 128], mdt, tag="dm")
        acc = io.tile([128, 8], f32, tag="acc")

        # consts first (gpsimd queue), zero the accumulator
        nc.gpsimd.dma_start(cst[:], cst_d.ap())
        nc.gpsimd.dma_start(dm[:], dmat_d.ap())
        nc.vector.memset(acc[:], 0.0)

        # fill unfilled stack gaps with junk DRAM data (consts are zero
        # there; vector memset can't start at unaligned partitions)
        for (hf, pa, pb, ce, ne) in plan["memset_specs"]:
            for ft, srcd in ((f1, p1d), (f2, p2d)):
                src = bass.AP(srcd, 0, [[BPC * W, pb - pa], [1, ne]])
                nc.scalar.dma_start(ft[pa:pb, ce:ce + ne], src)

        # field DMAs, half 0 first, engines rotated
        eng_rot = [nc.sync, nc.scalar, nc.gpsimd]
        specs = sorted(plan["dma_specs"], key=lambda t: t[0])
        ei = 0
        for (hf, soff, sdims, p0, P, ce, ne) in specs:
            for ft, srcd in ((f1, p1d), (f2, p2d)):
                src = bass.AP(srcd, soff, [list(d) for d in sdims])
                dst = ft[p0:p0 + P, ce:ce + ne]
                eng_rot[ei % len(eng_rot)].dma_start(dst, src)
                ei += 1

        A3 = cst[:, 0:Wp].unsqueeze(2).broadcast_to([128, Wp, BPC])
        B3 = cst[:, Wp:2 * Wp].unsqueeze(2).broadcast_to([128, Wp, BPC])
        M3 = cst[:, 2 * Wp:3 * Wp].unsqueeze(2).broadcast_to([128, Wp, BPC])

        for hi, (e0, e1) in enumerate(halves):
            x0, x1 = e0 // BPC, e1 // BPC
            # psi = 0.025*f2 - f1
            nc.vector.scalar_tensor_tensor(
                psi[:, e0:e1], f2[:, e0:e1], 0.025, f1[:, e0:e1],
                op0=mult, op1=subt)
            # dxs[e] = psi[e+2col] - psi[e]  (x-derivative at e+1col)
            nc.vector.tensor_sub(
                dxs[:, e0:e1 - 2 * BPC], psi[:, e0 + 2 * BPC:e1],
                psi[:, e0:e1 - 2 * BPC])
            nc.vector.memset(dxs[:, e1 - 2 * BPC:e1], 0.0)
            # dy via banded matmul, v = B * dy  (chunks of 512)
            for c0e in range(e0, e1, 512):
                c1e = min(c0e + 512, e1)
                ln = c1e - c0e
                ps = psum.tile([128, 512], f32, tag="dy")
                nc.tensor.matmul(ps[:, 0:ln], dm[:, :], psi[:, c0e:c1e],
                                 start=True, stop=True)
                xs0, xs1 = c0e // BPC, c1e // BPC
                nc.vector.tensor_mul(
                    v[:].rearrange("p (x b) -> p x b", b=BPC)[:, xs0:xs1, :],
                    ps[:, 0:ln].rearrange("p (x b) -> p x b", b=BPC),
                    B3[:, xs0:xs1, :])
            # u = A_sh * dxs
            nc.vector.tensor_mul(
                u[:].rearrange("p (x b) -> p x b", b=BPC)[:, x0:x1 - 1, :],
                dxs[:].rearrange("p (x b) -> p x b", b=BPC)[:, x0:x1 - 1, :],
                A3[:, x0:x1 - 1, :])
            # wt = u + v shifted one col
            nc.vector.tensor_add(wt[:, e0:e1 - BPC], u[:, e0:e1 - BPC],
                                 v[:, e0 + BPC:e1])
            # pot path on gpsimd
            nc.gpsimd.tensor_sub(df[:, e0:e1], f1[:, e0:e1], f2[:, e0:e1])
            nc.gpsimd.tensor_mul(
                w1[:].rearrange("p (x b) -> p x b", b=BPC)[:, x0:x1, :],
                df[:].rearrange("p (x b) -> p x b", b=BPC)[:, x0:x1, :],
                M3[:, x0:x1, :])
            # squares with free-dim accumulate
            nc.scalar.activation(scr[:, e0:e1 - BPC], wt[:, e0:e1 - BPC], SQ,
                                 accum_out=acc[:, 2 * hi:2 * hi + 1])
            nc.scalar.activation(scr[:, e0:e1 - BPC], w1[:, e0:e1 - BPC], SQ,
                                 accum_out=acc[:, 2 * hi + 1:2 * hi + 2])

        nc.sync.dma_start(acc_d.ap(), acc[:])

    nc.compile()
    return nc


_CACHE = {}


def kernel(output_in, output_out, interface_mask):
    from concourse.bass_utils import run_bass_kernel_spmd

    phi1 = np.asarray(output_in).reshape(B, H, W)
    phi2 = np.asarray(output_out).reshape(B, H, W)
    mask = np.asarray(interface_mask).astype(bool)

    n_mask = float(mask.sum())
    if n_mask == 0.0:
        return np.float32(np.nan)

    key = (mask.tobytes(), DEVICE_DTYPE)
    if key not in _CACHE:
        plan, host_cells, np_dt = _prepare(mask)
        nc = _build_nc(plan, DEVICE_DTYPE) if plan else None
        _CACHE[key] = (plan, host_cells, np_dt, nc)
    plan, host_cells, np_dt, nc = _CACHE[key]

    pot = der = 0.0
    if nc is not None:
        c1 = phi1.astype(np_dt)
        c2 = phi2.astype(np_dt)
        in_maps = []
        for c in range(N_CORES):
            sl = slice(c * BPC, (c + 1) * BPC)
            m = {
                "cst": plan["consts"],
                "dmat": plan["dmat"],
                "p1": np.ascontiguousarray(c1[sl].transpose(1, 2, 0)).ravel(),
                "p2": np.ascontiguousarray(c2[sl].transpose(1, 2, 0)).ravel(),
            }
            in_maps.append(m)
        res = run_bass_kernel_spmd(
            nc, in_maps, core_ids=list(range(N_CORES)), trace=TRACE
        )
        global LAST_EXEC_NS
        LAST_EXEC_NS = res.exec_time_ns
        for r in res.results:
            a = r["acc"].astype(np.float64)
            der += float(a[:, 0].sum() + a[:, 2].sum())
            pot += float(a[:, 1].sum() + a[:, 3].sum())

    if host_cells.any():
        nx, ny = _normals(H, W)
        hp, hd = _host_contrib(np.nonzero(host_cells), phi1, phi2, nx, ny)
        pot += hp
        der += hd

    denom = B * n_mask
    return np.float32(WEIGHT * (pot + der) / denom)


# revision 16
# speedup vs baseline: 4.1120x; 1.2808x over previous
"""InterfaceBoundaryLoss Trainium2 kernel.

Data-parallel over batch across 8 NeuronCores; each core owns 8 batches.

The [H,W] interface mask (a thin circle) is analyzed on the host and covered
with "segments": sheared bands of K*8 consecutive image rows whose column
window slides by an integer shear per 8-row group (vertical segments), or
K column-chunks of one 8-row band (horizontal segments, for flat/wide
stretches).  Either way one segment is ONE strided DMA whose destination is
a plain [K*8 partitions, 8w cols] SBUF slice.  Segments are stacked into the
128 partitions; all segments together form a single packed [128, E] layout
(E = packed_cols * 8 batches, batch innermost).  Host uploads phi in
batch-inner [H, W, 8] layout so each DMA line covers all 8 batches.

All compute runs as a handful of full-width instructions:
  psi  = 0.025*phi2 - phi1                       (Vector, one stt)
  dxs  = psi[x+1] - psi[x-1]   (shifted sub)     (Vector, one sub)
  dy   = banded 128x128 matmul over partitions   (PE, E/512 matmuls)
  u    = A_sh * dxs ; v = B * dy ; wt = u + v>>1 (Vector)
  df   = phi1 - phi2 ; w1 = M * df               (GpSimd)
  der  = sum(wt^2), pot = sum(w1^2)              (Scalar activation accum)
Segment margins carry A=B=M=0 so stencil reads crossing segment/slot
boundaries are multiplied by zero; every element of the packed region is
either real (finite) DRAM data or memset, so no NaNs reach the matmul.
Work is split into two halves so compute overlaps the tail DMAs.

Mask cells on the frame border (edge-padding semantics) and any cells the
planner cannot cover are computed exactly on the host (none for the circle).
"""

import sys

for _p in ("/opt/trn_rl_repo",):
    if _p not in sys.path:
        sys.path.append(_p)

import numpy as np
import ml_dtypes

B, H, W = 64, 1024, 1024
EPS1, EPS2 = 80.0, 2.0
DX, DY = 0.001, 0.001
CX, CY = 512.0, 512.0
WEIGHT = 1.0
N_CORES = 8
BPC = B // N_CORES  # 8 batches per core, innermost in device layout

# "bf16" or "f32" device compute dtype.
DEVICE_DTYPE = "bf16"

# set TRACE=True (e.g. from a test harness) to profile the device run;
# LAST_EXEC_NS then holds the measured NEFF execution time.
TRACE = False
LAST_EXEC_NS = None

GR = 8            # rows loaded per box (partition granularity)
OSTEP = 6         # owned rows per vertical box (boxes overlap by 2)
ROW_GAP = 8       # col gap that splits a row into two intervals
CHAIN_SLACK = 24  # max col distance to keep chaining rows
CHOPW = 28        # box window wider than this -> horizontal chop
WCHUNK = 16       # owned cols per horizontal chunk


def _normals(h, w):
    ii = np.arange(h, dtype=np.float64)[:, None]
    jj = np.arange(w, dtype=np.float64)[None, :]
    nx = jj - CX
    ny = ii - CY
    norm = np.sqrt(nx * nx + ny * ny)
    safe = np.where(norm > 0, norm, 1.0)
    return nx / safe, ny / safe


def _intervals(cols, gap):
    out = []
    s = p = cols[0]
    for c in cols[1:]:
        if c - p > gap:
            out.append((s, p))
            s = c
        p = c
    out.append((s, p))
    return out


class _Seg:
    """K stacked 8-row windows (boxes), one DMA.

    vertical (rstep=OSTEP): box g loads rows [r0+6g, r0+6g+8),
        cols [c0+g*dlt, +w).  Box g owns rows [r0+6g+1, r0+6g+7).
    horizontal (rstep=0): all boxes load rows [r0, r0+8),
        box g cols [c0+g*dlt, +w).  Owned rows [r0+1, r0+7).
    Every box is vertically self-contained (Dy stencil stays inside its
    8 loaded rows), so the per-box column window may shear freely.
    """
    __slots__ = ("r0", "K", "c0", "dlt", "w", "rstep", "rows", "slot",
                 "s0", "off", "w_load", "half")

    def __init__(self, r0, K, c0, dlt, w, rstep, rows):
        self.r0, self.K, self.c0 = int(r0), int(K), int(c0)
        self.dlt, self.w, self.rstep = int(dlt), int(w), int(rstep)
        self.rows = rows  # dict row -> (lo, hi) owned cells of this seg


def _plan(mask):
    """Cover interior mask cells with segments."""
    h, w_ = mask.shape
    border = np.zeros_like(mask)
    border[0, :] = border[-1, :] = True
    border[:, 0] = border[:, -1] = True
    host_cells = mask & border
    core = mask & ~border

    rows_any = np.flatnonzero(core.any(axis=1))
    if rows_any.size == 0:
        return [], host_cells

    # per-row intervals
    row_iv = {}
    for r in rows_any:
        cols = np.flatnonzero(core[r])
        for iv in _intervals(cols, ROW_GAP):
            row_iv.setdefault(r, []).append(iv)

    # chains: link intervals across consecutive rows
    chains = []      # each: list of (r, lo, hi)
    active = {}      # chain idx -> (r, lo, hi)
    for r in rows_any:
        newactive = {}
        used = set()
        for (lo, hi) in row_iv[r]:
            best, bestov = None, None
            for ci, (pr, plo, phi) in active.items():
                if ci in used or pr != r - 1:
                    continue
                ov = min(hi, phi) - max(lo, plo)
                if ov >= -CHAIN_SLACK and (bestov is None or ov > bestov):
                    best, bestov = ci, ov
            if best is None:
                chains.append([])
                best = len(chains) - 1
            chains[best].append((int(r), int(lo), int(hi)))
            used.add(best)
            newactive[best] = (int(r), int(lo), int(hi))
        active = newactive

    segs = []

    def chop(r0, rows):
        """Horizontal chop of rows [r0+1, r0+7) into WCHUNK-col chunks."""
        allc = sorted(set(c for (lo, hi) in rows.values() for c in (lo, hi)))
        lo = min(lo for (lo, hi) in rows.values())
        hi = max(hi for (lo, hi) in rows.values())
        cw = hi - lo + 1
        K = -(-cw // WCHUNK)
        out = []
        for k0 in range(0, K, 16):
            kk = min(16, K - k0)
            rws = {}
            for r, (l, hh) in rows.items():
                a = max(l, lo + k0 * WCHUNK)
                b = min(hh, lo + (k0 + kk) * WCHUNK - 1)
                if a <= b:
                    rws[r] = (a, b)
            out.append(_Seg(r0, kk, lo + k0 * WCHUNK - 1, WCHUNK,
                            WCHUNK + 2, 0, rws))
        return out

    def win_fit(gwins):
        K = len(gwins)
        good = [i for i, wn in enumerate(gwins) if wn is not None]
        los = np.array([gwins[i][0] for i in good], dtype=np.int64)
        his = np.array([gwins[i][1] for i in good], dtype=np.int64)
        if len(good) > 1:
            d = int(round((los[-1] - los[0]) / (good[-1] - good[0])))
        else:
            d = 0
        gi = np.array(good)
        c0 = int((los - 1 - gi * d).min())
        w = int((his + 2 - (c0 + gi * d)).max() + 1)
        return c0, d, w

    def seg_windows(ch_rows, lo_r, hi_r):
        """Box windows for owned rows [lo_r, hi_r); box g owns 6 rows
        [lo_r + 6g, +6) and loads them plus one margin row above/below."""
        K = -(-(hi_r - lo_r) // OSTEP)
        wins = []
        for g in range(K):
            lo = hi = None
            ra = lo_r + OSTEP * g
            for r in range(ra, min(ra + OSTEP, hi_r)):
                iv = ch_rows.get(r)
                if iv is None:
                    continue
                lo = iv[0] if lo is None else min(lo, iv[0])
                hi = iv[1] if hi is None else max(hi, iv[1])
            wins.append(None if lo is None else (lo, hi))
        return K, wins

    def segcost(w, K):
        return 600.0 + 40.0 * w * max(K, 4) / 16.0

    def emit(ch_rows, lo_r, hi_r):
        K, wins = seg_windows(ch_rows, lo_r, hi_r)
        c0, d, w = win_fit(wins)
        nown = hi_r - lo_r
        if K == 1:
            if w > CHOPW + 4:
                rws = {r: ch_rows[r] for r in range(lo_r, hi_r)
                       if r in ch_rows}
                return chop(lo_r - 1, rws)
            return [_Seg(lo_r - 1, K, c0, d, w, OSTEP,
                         {r: ch_rows[r] for r in range(lo_r, hi_r)
                          if r in ch_rows})]
        # try split at the middle box boundary
        mid = lo_r + OSTEP * (K // 2)
        Kl, wl_ = seg_windows(ch_rows, lo_r, mid)
        Kr, wr_ = seg_windows(ch_rows, mid, hi_r)
        _, _, wl = win_fit(wl_)
        _, _, wr = win_fit(wr_)
        if (w > CHOPW + 4) or (segcost(wl, Kl) + segcost(wr, Kr)
                               < segcost(w, K)):
            return emit(ch_rows, lo_r, mid) + emit(ch_rows, mid, hi_r)
        return [_Seg(lo_r - 1, K, c0, d, w, OSTEP,
                     {r: ch_rows[r] for r in range(lo_r, hi_r)
                      if r in ch_rows})]

    for ch in chains:
        ch_rows = {r: (lo, hi) for (r, lo, hi) in ch}
        rs = sorted(ch_rows)
        start = 0
        for i in range(1, len(rs) + 1):
            if i == len(rs) or rs[i] != rs[i - 1] + 1:
                lo_r, hi_r = rs[start], rs[i - 1] + 1
                while lo_r < hi_r:
                    top = min(lo_r + 16 * OSTEP, hi_r)
                    segs.extend(emit(ch_rows, lo_r, top))
                    lo_r = top
                start = i
    return segs, host_cells


def _host_contrib(cells_ij, phi1, phi2, nx, ny):
    if len(cells_ij[0]) == 0:
        return 0.0, 0.0
    ii, jj = cells_ij
    p1 = phi1.astype(np.float64)
    p2 = phi2.astype(np.float64)
    d = p1[:, ii, jj] - p2[:, ii, jj]
    pot = float(np.sum(d * d))
    jc = np.clip(jj, 1, W - 2)
    ic = np.clip(ii, 1, H - 2)

    def dn(p):
        dpx = (p[:, ii, jc + 1] - p[:, ii, jc - 1]) / (2.0 * DX)
        dpy = (p[:, ic + 1, jj] - p[:, ic - 1, jj]) / (2.0 * DY)
        return nx[ii, jj] * dpx + ny[ii, jj] * dpy

    mm = EPS1 * dn(p1) - EPS2 * dn(p2)
    der = float(np.sum(mm * mm))
    return pot, der


def _prepare(mask):
    """Plan the cover and build packed const arrays + DMA specs."""
    segs, host_cells = _plan(mask)
    np_dt = ml_dtypes.bfloat16 if DEVICE_DTYPE == "bf16" else np.float32
    if not segs:
        return None, host_cells, np_dt

    nx, ny = _normals(H, W)
    af = 40000.0 * nx
    bf = 40000.0 * ny

    # shelf-pack into slots (16 stacks of GR partitions)
    slots = []  # [span, stacks_used, [segs]]
    drop = []
    for s in sorted(segs, key=lambda s: -s.w):
        placed = False
        for si, sl in enumerate(slots):
            if sl[1] + s.K <= 16 and s.w <= sl[0]:
                s.slot = si
                s.s0 = sl[1]
                sl[1] += s.K
                sl[2].append(s)
                placed = True
                break
        if not placed:
            s.slot = len(slots)
            s.s0 = 0
            slots.append([s.w, s.K, [s]])

    Wp = sum(sl[0] for sl in slots)
    off = 0
    for sl in slots:
        for s in sl[2]:
            s.off = off
            s.w_load = sl[0]
        off += sl[0]

    # halves split at a slot boundary near Wp/2
    acc = 0
    half_split = len(slots)
    for si, sl in enumerate(slots):
        if acc >= Wp / 2:
            half_split = si
            break
        acc += sl[0]
    wp1 = sum(sl[0] for sl in slots[:half_split])
    if wp1 in (0, Wp):
        wp1 = Wp
    for si, sl in enumerate(slots):
        for s in sl[2]:
            s.half = 0 if si < half_split else 1

    A = np.zeros((128, Wp), dtype=np.float64)
    Bc = np.zeros((128, Wp), dtype=np.float64)
    M = np.zeros((128, Wp), dtype=np.float64)
    filled = np.zeros((128, Wp), dtype=bool)

    borderm = np.zeros_like(mask)
    borderm[0, :] = borderm[-1, :] = True
    borderm[:, 0] = borderm[:, -1] = True
    core = mask & ~borderm

    extra_host = np.zeros_like(mask)
    dma_specs = []  # (half, src_off, src_dims, p0, P, col0_e, ncol_e)
    for s in segs:
        # bounds clamp for extended load width
        cmax = max(s.c0 + g * s.dlt for g in range(s.K))
        cmin = min(s.c0 + g * s.dlt for g in range(s.K))
        shift = 0
        if cmax + s.w_load > W:
            shift = cmax + s.w_load - W
        if cmin - shift < 0:
            # cannot satisfy bounds: push cells to host
            for r, (lo, hi) in s.rows.items():
                extra_host[r, lo:hi + 1] = True
            continue
        s.c0 -= shift

        rmax_load = s.r0 + (s.K - 1) * s.rstep + GR
        assert s.r0 >= 0 and rmax_load <= H

        ok = True
        if s.rstep != 0:
            for r, (lo, hi) in s.rows.items():
                g = (r - s.r0 - 1) // OSTEP
                cg = s.c0 + g * s.dlt
                if not (0 <= g < s.K and cg + 1 <= lo
                        and hi <= cg + s.w - 2):
                    ok = False
                    break
        if not ok:
            for r, (lo, hi) in s.rows.items():
                extra_host[r, lo:hi + 1] = True
            continue

        for r, (lo, hi) in s.rows.items():
            if s.rstep == 0:
                gs = range(s.K)
                rel = r - s.r0
                assert 1 <= rel <= GR - 2
            else:
                gs = [(r - s.r0 - 1) // OSTEP]
            for g in gs:
                cg = s.c0 + g * s.dlt
                if s.rstep != 0:
                    rel = r - (s.r0 + g * OSTEP)
                    assert 1 <= rel <= GR - 2
                llo, lhi = max(lo, cg + 1), min(hi, cg + s.w - 2)
                for c in range(llo, lhi + 1):
                    if not core[r, c]:
                        continue
                    p = (s.s0 + g) * GR + rel
                    x = s.off + (c - cg)
                    assert not filled[p, x]
                    filled[p, x] = True
                    M[p, x] = 1.0
                    Bc[p, x] = bf[r, c]
                    A[p, x - 1] = af[r, c]

        # both fields interleaved in DRAM: elem (r, c, fld, b) at
        # ((r*W + c)*2 + fld)*BPC;  a line covers both fields
        IV = 2 * BPC
        sstep = (s.rstep * W + s.dlt) * IV
        src_off = (s.r0 * W + s.c0) * IV
        if s.K > 1:
            src_dims = [[sstep, s.K], [IV * W, GR], [1, IV * s.w_load]]
        else:
            src_dims = [[IV * W, GR], [1, IV * s.w_load]]
        dma_specs.append((s.half, src_off, src_dims, s.s0 * GR, s.K * GR,
                          s.off * IV, s.w_load * IV))

    if extra_host.any():
        host_cells = host_cells | extra_host

    # memset unfilled stacks of each slot
    memset_specs = []
    off = 0
    for si, sl in enumerate(slots):
        span, used, _ = sl
        if used < 16:
            memset_specs.append((0 if si < half_split else 1,
                                 used * GR, 128, off * BPC, span * BPC))
        off += span

    consts = np.concatenate([A, Bc, M], axis=1).astype(np_dt)
    dmat = np.zeros((128, 128), dtype=np.float64)
    for mi in range(1, 127):
        dmat[mi + 1, mi] = 1.0
        dmat[mi - 1, mi] = -1.0

    plan = {
        "Wp": Wp, "wp1": wp1, "E": Wp * BPC, "E1": wp1 * BPC,
        "dma_specs": dma_specs, "memset_specs": memset_specs,
        "consts": consts, "dmat": dmat.astype(np_dt), "segs": segs,
    }
    return plan, host_cells, np_dt


def _build_nc(plan, dt_str):
    from contextlib import ExitStack
    from concourse import bass, bacc, tile, mybir

    mdt = mybir.dt.bfloat16 if dt_str == "bf16" else mybir.dt.float32
    f32 = mybir.dt.float32
    mult = mybir.AluOpType.mult
    subt = mybir.AluOpType.subtract
    SQ = mybir.ActivationFunctionType.Square

    Wp, E, E1 = plan["Wp"], plan["E"], plan["E1"]
    halves = [(0, E1), (E1, E)] if E1 not in (0, E) else [(0, E)]

    nc = bacc.Bacc(
        "TRN2", target_bir_lowering=False, debug=False, num_devices=N_CORES
    )
    ppd = nc.dram_tensor("pp", [H * W * 2 * BPC], mdt, kind="ExternalInput")
    cst_d = nc.dram_tensor("cst", [128, 3 * Wp], mdt, kind="ExternalInput")
    dmat_d = nc.dram_tensor("dmat", [128, 128], mdt, kind="ExternalInput")
    acc_d = nc.dram_tensor("acc", [128, 8], f32, kind="ExternalOutput")

    with tile.TileContext(nc) as tc, ExitStack() as ctx:
        io = ctx.enter_context(tc.tile_pool(name="io", bufs=1))
        psum = ctx.enter_context(tc.tile_pool(name="ps", bufs=4, space="PSUM"))

        ff = io.tile([128, 2 * E], mdt, tag="ff")
        psi = io.tile([128, E], mdt, tag="psi")
        dxs = io.tile([128, E], mdt, tag="dxs")
        u = io.tile([128, E], mdt, tag="u")
        v = io.tile([128, E], mdt, tag="v")
        wt = io.tile([128, E], mdt, tag="wt")
        df = io.tile([128, E], mdt, tag="df")
        w1 = io.tile([128, E], mdt, tag="w1")
        scr = io.tile([128, E], mdt, tag="scr")
        cst = io.tile([128, 3 * Wp], mdt, tag="cst")
        dm = io.tile([128, 128], mdt, tag="dm")
        acc = io.tile([128, 8], f32, tag="acc")

        # consts first (gpsimd queue), zero the accumulator
        nc.gpsimd.dma_start(cst[:], cst_d.ap())
        nc.gpsimd.dma_start(dm[:], dmat_d.ap())
        nc.vector.memset(acc[:], 0.0)

        # fill unfilled stack gaps with junk DRAM data (consts are zero
        # there; vector memset can't start at unaligned partitions)
        for (hf, pa, pb, ce, ne) in plan["memset_specs"]:
            src = bass.AP(ppd, 0, [[2 * BPC * W, pb - pa], [1, 2 * ne]])
            nc.scalar.dma_start(ff[pa:pb, 2 * ce:2 * ce + 2 * ne], src)

        # field DMAs, half 0 first, engines rotated
        eng_rot = [nc.sync, nc.scalar, nc.gpsimd]
        specs = sorted(plan["dma_specs"], key=lambda t: t[0])
        for ei, (hf, soff, sdims, p0, P, ce, ne) in enumerate(specs):
            src = bass.AP(ppd, soff, [list(d) for d in sdims])
            dst = ff[p0:p0 + P, ce:ce + ne]
            eng_rot[ei % len(eng_rot)].dma_start(dst, src)

        ff4 = ff[:].rearrange("p (x f b) -> p x f b", f=2, b=BPC)
        f1v = ff4[:, :, 0, :]
        f2v = ff4[:, :, 1, :]

        A3 = cst[:, 0:Wp].unsqueeze(2).broadcast_to([128, Wp, BPC])
        B3 = cst[:, Wp:2 * Wp].unsqueeze(2).broadcast_to([128, Wp, BPC])
        M3 = cst[:, 2 * Wp:3 * Wp].unsqueeze(2).broadcast_to([128, Wp, BPC])

        for hi, (e0, e1) in enumerate(halves):
            x0, x1 = e0 // BPC, e1 // BPC
            psi3 = psi[:].rearrange("p (x b) -> p x b", b=BPC)
            df3 = df[:].rearrange("p (x b) -> p x b", b=BPC)
            # psi = 0.025*f2 - f1
            nc.vector.scalar_tensor_tensor(
                psi3[:, x0:x1, :], f2v[:, x0:x1, :], 0.025, f1v[:, x0:x1, :],
                op0=mult, op1=subt)
            # dxs[e] = psi[e+2col] - psi[e]  (x-derivative at e+1col)
            nc.vector.tensor_sub(
                dxs[:, e0:e1 - 2 * BPC], psi[:, e0 + 2 * BPC:e1],
                psi[:, e0:e1 - 2 * BPC])
            nc.vector.memset(dxs[:, e1 - 2 * BPC:e1], 0.0)
            # dy via banded matmul, v = B * dy  (chunks of 512)
            for c0e in range(e0, e1, 512):
                c1e = min(c0e + 512, e1)
                ln = c1e - c0e
                ps = psum.tile([128, 512], f32, tag="dy")
                nc.tensor.matmul(ps[:, 0:ln], dm[:, :], psi[:, c0e:c1e],
                                 start=True, stop=True)
                xs0, xs1 = c0e // BPC, c1e // BPC
                nc.vector.tensor_mul(
                    v[:].rearrange("p (x b) -> p x b", b=BPC)[:, xs0:xs1, :],
                    ps[:, 0:ln].rearrange("p (x b) -> p x b", b=BPC),
                    B3[:, xs0:xs1, :])
            # u = A_sh * dxs
            nc.vector.tensor_mul(
                u[:].rearrange("p (x b) -> p x b", b=BPC)[:, x0:x1 - 1, :],
                dxs[:].rearrange("p (x b) -> p x b", b=BPC)[:, x0:x1 - 1, :],
                A3[:, x0:x1 - 1, :])
            # wt = u + v shifted one col
            nc.vector.tensor_add(wt[:, e0:e1 - BPC], u[:, e0:e1 - BPC],
                                 v[:, e0 + BPC:e1])
            # pot path: df on vector, mask-mul on gpsimd
            nc.vector.tensor_sub(df3[:, x0:x1, :], f1v[:, x0:x1, :],
                                 f2v[:, x0:x1, :])
            nc.gpsimd.tensor_mul(
                w1[:].rearrange("p (x b) -> p x b", b=BPC)[:, x0:x1, :],
                df3[:, x0:x1, :],
                M3[:, x0:x1, :])
            # squares with free-dim accumulate
            nc.scalar.activation(scr[:, e0:e1 - BPC], wt[:, e0:e1 - BPC], SQ,
                                 accum_out=acc[:, 2 * hi:2 * hi + 1])
            nc.scalar.activation(scr[:, e0:e1 - BPC], w1[:, e0:e1 - BPC], SQ,
                                 accum_out=acc[:, 2 * hi + 1:2 * hi + 2])

        nc.sync.dma_start(acc_d.ap(), acc[:])

    nc.compile()
    return nc


_CACHE = {}


def kernel(output_in, output_out, interface_mask):
    from concourse.bass_utils import run_bass_kernel_spmd

    phi1 = np.asarray(output_in).reshape(B, H, W)
    phi2 = np.asarray(output_out).reshape(B, H, W)
    mask = np.asarray(interface_mask).astype(bool)

    n_mask = float(mask.sum())
    if n_mask == 0.0:
        return np.float32(np.nan)

    key = (mask.tobytes(), DEVICE_DTYPE)
    if key not in _CACHE:
        plan, host_cells, np_dt = _prepare(mask)
        nc = _build_nc(plan, DEVICE_DTYPE) if plan else None
        _CACHE[key] = (plan, host_cells, np_dt, nc)
    plan, host_cells, np_dt, nc = _CACHE[key]

    pot = der = 0.0
    if nc is not None:
        c1 = phi1.astype(np_dt)
        c2 = phi2.astype(np_dt)
        in_maps = []
        for c in range(N_CORES):
            sl = slice(c * BPC, (c + 1) * BPC)
            pp = np.stack([c1[sl].transpose(1, 2, 0),
                           c2[sl].transpose(1, 2, 0)], axis=2)
            m = {
                "cst": plan["consts"],
                "dmat": plan["dmat"],
                "pp": np.ascontiguousarray(pp).ravel(),
            }
            in_maps.append(m)
        res = run_bass_kernel_spmd(
            nc, in_maps, core_ids=list(range(N_CORES)), trace=TRACE
        )
        global LAST_EXEC_NS
        LAST_EXEC_NS = res.exec_time_ns
        for r in res.results:
            a = r["acc"].astype(np.float64)
            der += float(a[:, 0].sum() + a[:, 2].sum())
            pot += float(a[:, 1].sum() + a[:, 3].sum())

    if host_cells.any():
        nx, ny = _normals(H, W)
        hp, hd = _host_contrib(np.nonzero(host_cells), phi1, phi2, nx, ny)
        pot += hp
        der += hd

    denom = B * n_mask
    return np.float32(WEIGHT * (pot + der) / denom)
